# revision 12
# baseline (speedup 1.0000x reference)
"""Trainium2 Bass kernel for nn_MixedDecoder (moe_routing).

Math (matches the reference exactly): only the LAST expert layer matters —
the reference never feeds layer outputs back into `z`, so layers 0/1 are
dead code.  Computed per sample b:
    coef = softmax(gate_mlp(z))                        # [B, 8]
    out  = sum_e coef[b,e] * (z @ w2[e] + b2[e])       # [B, 256]

Sharding: data-parallel over batch B=2048 across 8 cores (256 rows/core),
weights replicated.  All matmul operands are bf16 (halves HBM traffic vs
fp32; PE rate is unchanged at one moving column per cycle).  Every bias is
folded into a matmul via an extra ones-row on the stationary side:
 - b2[e] rides as row 96 of the last K-chunk of the packed w2,
 - g0_b as row 96 of the last K-chunk of g0_w,
 - the ELU "+1" adjustments (b1_adj, adj2) as row 64 of g1_w / g2_w, paired
   with a ones-row in the relu-piece tiles (zeros-row in the exp-piece).
ELU itself is relu(x) + min(exp(x),1) with the "+1" absorbed as above.
Expert matmuls keep zT chunks stationary with expert pairs side-by-side as
[K, 512] moving operands; per-expert coefficient scaling (unnormalized
softmax numerator) happens on PSUM eviction into pair tiles (ACT and DVE
alternating), then one PE accumulation of 4 identity-matmuls re-sums the 8
scaled tiles; the final eviction applies the 1/sum normalization.
"""

import numpy as np

N_CORES = 8
B = 2048
IN_SIZE = 288
HIDDEN = 256
E = 8
GATE_H = 64
OUT_SIZE = 256
BL = B // N_CORES          # 256 rows per core
NCH = BL // 128            # 2 batch chunks of 128
KC = 96                    # K chunk size (288 = 3 x 96)
NK = IN_SIZE // KC
KP = KC + 1                # 97: chunk rows + shared ones/bias row
PW = NK * 2 * OUT_SIZE     # 1536 cols per expert-pair piece of packed w2

_CACHE = {}


def _build_nc(reps=1, variant="full"):
    from concourse import bacc
    import concourse.mybir as mybir
    from concourse.tile import TileContext
    from concourse.masks import make_identity

    dt = mybir.dt
    F32 = dt.float32
    F32R = dt.float32r
    BF16 = dt.bfloat16
    AF = mybir.ActivationFunctionType
    OP = mybir.AluOpType

    nc = bacc.Bacc("TRN2", target_bir_lowering=False, debug=False)

    # packed inputs (see make_in_maps).  zTp row 96 = ones (bias row).
    zT_d = nc.declare_dram_parameter("zTp", [KP, NK * BL], BF16, isOutput=False)
    # gate pack: cols 0:192 = g0_w K-chunks (row 96 of chunk 2 = g0_b);
    # cols 192:256 rows 0:65 = [g1_w; b1_adj]; cols 256:264 = [g2_w; adj2]
    GWX = NK * GATE_H + GATE_H + E
    gw_d = nc.declare_dram_parameter("gwp", [KP, GWX], BF16, isOutput=False)
    # w2 pack, pair-major: piece p = [97, 3*512]; K-chunk i block rows 0:96 =
    # w2.transpose(1,0,2)[i*96:(i+1)*96, pair cols]; row 96 of block 2 = b2
    w2_d = nc.declare_dram_parameter("w2p", [KP, E // 2 * PW], BF16,
                                     isOutput=False)
    out_d = nc.declare_dram_parameter("outp", [128, NCH * OUT_SIZE], BF16,
                                      isOutput=True)

    with TileContext(nc) as tc:
      with tc.tile_pool(name="const", bufs=1) as cp:
        ident = cp.tile([128, 128], F32, name="ident")
        make_identity(nc, ident[:])
        ident_r = cp.tile([128, 128], F32R, name="identr")
        nc.vector.tensor_copy(ident_r[:], ident[:])

        # dummy exp so the ACT Exp-table load happens before it's needed
        warm = cp.tile([1, 1], F32, name="warm")
        nc.vector.memset(warm[:], 0.0)
        warm2 = cp.tile([1, 1], F32, name="warm2")
        nc.scalar.activation(warm2[:], warm[:], AF.Exp)

        with (
            tc.tile_pool(name="inp", bufs=2) as ip,
            tc.tile_pool(name="w2p", bufs=2) as wp,
            tc.tile_pool(name="wk", bufs=2) as wk,
            tc.tile_pool(name="py", bufs=4, space="PSUM") as py,
            tc.tile_pool(name="pb", bufs=2, space="PSUM") as pb,
            tc.tile_pool(name="pg", bufs=2, space="PSUM") as pg,
        ):
          for _rep in range(reps):
            # -------- DMAs: gate-critical first (zT, gate weights) ----------
            zT_r = ip.tile([KP, NK * BL], BF16, name="zT")
            gw_r = ip.tile([KP, GWX], BF16, name="gwr")
            w2_r = wp.tile([KP, E // 2 * PW], BF16, name="w2r")
            if variant != "compute_only":
                nc.sync.dma_start(out=zT_r[:], in_=zT_d.ap())
                nc.sync.dma_start(out=gw_r[:], in_=gw_d.ap())
                for p in range(E // 2):
                    nc.sync.dma_start(out=w2_r[:, p * PW:(p + 1) * PW],
                                      in_=w2_d.ap()[:, p * PW:(p + 1) * PW])
            if variant == "dma_only":
                out_sb0 = wk.tile([128, NCH * OUT_SIZE], BF16, name="outsb")
                for c in range(NCH):
                    nc.sync.dma_start(
                        out=out_d.ap()[:, c * OUT_SIZE:(c + 1) * OUT_SIZE],
                        in_=out_sb0[:, c * OUT_SIZE:(c + 1) * OUT_SIZE])
                continue

            g1w_r = gw_r[0:GATE_H + 1, NK * GATE_H:NK * GATE_H + GATE_H]
            g2w_r = gw_r[0:GATE_H + 1, NK * GATE_H + GATE_H:GWX]

            if _rep == 0:
                # PE warm-up: dummy matmuls so the HAM clock-gate releases
                # before the real work arrives (throwaway results)
                wu_ps = pb.tile([128, 128], F32, name="wups", tag="pb")
                for _ in range(8):
                    nc.tensor.matmul(wu_ps[:], ident_r[:], ident_r[:],
                                     start=True, stop=True)

            # Gate MLP in transposed layout.  ELU is kept as two pieces
            # (relu(x) and min(exp(x),1), i.e. elu(x)+1 split) and the sum is
            # folded into the NEXT layer's matmul as two PSUM-accumulating
            # matmuls; bias offsets ride the ones-row of the relu piece.
            def elu_pieces(ps_in, pref):
                # [65, 256] tiles: row 64 = 1.0 in the relu piece (activates
                # the bias row of the next layer's weights), 0.0 in the exp
                # piece (so the bias is added exactly once)
                t_exp = wk.tile([GATE_H + 1, BL], BF16, name=f"{pref}_exp")
                nc.scalar.activation(t_exp[0:GATE_H, :], ps_in, AF.Exp)
                t_min = wk.tile([GATE_H + 1, BL], BF16, name=f"{pref}_min")
                nc.vector.tensor_scalar(t_min[0:GATE_H, :], t_exp[0:GATE_H, :],
                                        1.0, None, OP.min)
                t_relu = wk.tile([GATE_H + 1, BL], BF16, name=f"{pref}_relu")
                nc.vector.tensor_scalar(t_relu[0:GATE_H, :], ps_in,
                                        0.0, None, OP.max)
                nc.vector.memset(t_relu[GATE_H:GATE_H + 1, :], 1.0)
                return t_relu, t_min

            with tc.high_priority():
                h0_ps = pg.tile([GATE_H, BL], F32, name="h0ps", tag="pg")
                for i in range(NK):
                    rows = KP if i == NK - 1 else KC
                    nc.tensor.matmul(h0_ps[:],
                                     gw_r[0:rows, i * GATE_H:(i + 1) * GATE_H],
                                     zT_r[0:rows, i * BL:(i + 1) * BL],
                                     start=(i == 0), stop=(i == NK - 1))
                h0_a, h0_b = elu_pieces(h0_ps[:], "e0")

                h1_ps = pg.tile([GATE_H, BL], F32, name="h1ps", tag="pg")
                nc.tensor.matmul(h1_ps[:], g1w_r, h0_a[:], start=True, stop=False)
                nc.tensor.matmul(h1_ps[:], g1w_r[0:GATE_H, :], h0_b[0:GATE_H, :],
                                 start=False, stop=True)
                h1_a, h1_b = elu_pieces(h1_ps[:], "e1")

                # exp(logits) in [b, 8] layout per chunk for per-partition
                # scales (unnormalized; 1/sum is applied at final eviction)
                exp_sb = []    # (expc [128,8], rcp [128,1]) per chunk
                for c in range(NCH):
                    lg_ps = pg.tile([128, E], F32, name="lgps", tag="pg")
                    nc.tensor.matmul(lg_ps[:],
                                     h1_a[:, c * 128:(c + 1) * 128],
                                     g2w_r, start=True, stop=False)
                    nc.tensor.matmul(lg_ps[:],
                                     h1_b[0:GATE_H, c * 128:(c + 1) * 128],
                                     g2w_r[0:GATE_H, :], start=False, stop=True)
                    expc = wk.tile([128, E], F32, name="expc")
                    sume = wk.tile([128, 1], F32, name="sume")
                    nc.scalar.activation(expc[:], lg_ps[:], AF.Exp,
                                         accum_out=sume[:])
                    rcp = wk.tile([128, 1], F32, name="rcp")
                    nc.vector.reciprocal(rcp[:], sume[:])
                    exp_sb.append((expc, rcp))

            # ---------------- expert layer + combine ----------------
            # Y_pair matmuls -> coef-scaled fp32r eviction into pair tiles ->
            # PE re-sum: one PSUM accumulation of 4 identity-matmuls per chunk.
            out_sb = wk.tile([128, NCH * OUT_SIZE], BF16, name="outsb")
            for c in range(NCH):
                ys = []
                for p in range(E // 2):
                    yp = py.tile([128, 2 * OUT_SIZE], F32, name=f"yp{p}", tag="py")
                    for i in range(NK):
                        rows = KP if i == NK - 1 else KC
                        col0 = p * PW + i * 2 * OUT_SIZE
                        nc.tensor.matmul(
                            yp[:],
                            zT_r[0:rows, i * BL + c * 128:i * BL + (c + 1) * 128],
                            w2_r[0:rows, col0:col0 + 2 * OUT_SIZE],
                            start=(i == 0), stop=(i == NK - 1))
                    t = wk.tile([128, 2 * OUT_SIZE], F32R, name=f"ysp{p}")
                    for h in range(2):
                        e = 2 * p + h
                        src = yp[:, h * OUT_SIZE:(h + 1) * OUT_SIZE]
                        dst = t[:, h * OUT_SIZE:(h + 1) * OUT_SIZE]
                        scale = exp_sb[c][0][:, e:e + 1]
                        if h == 0:
                            nc.scalar.activation(dst, src, AF.Copy, scale=scale)
                        else:
                            nc.vector.tensor_scalar(dst, src, scale, None, OP.mult)
                    ys.append(t)

                out_ps = pb.tile([128, OUT_SIZE], F32, name="outps", tag="pb")
                for e in range(E):
                    src = ys[e // 2][:, (e % 2) * OUT_SIZE:(e % 2 + 1) * OUT_SIZE]
                    nc.tensor.matmul(out_ps[:], ident_r[:], src,
                                     start=(e == 0), stop=(e == E - 1))
                # final eviction applies the softmax normalization 1/sum
                nc.scalar.activation(out_sb[:, c * OUT_SIZE:(c + 1) * OUT_SIZE],
                                     out_ps[:], AF.Copy, scale=exp_sb[c][1][:])
                nc.sync.dma_start(
                    out=out_d.ap()[:, c * OUT_SIZE:(c + 1) * OUT_SIZE],
                    in_=out_sb[:, c * OUT_SIZE:(c + 1) * OUT_SIZE])

    nc.finalize()
    return nc


def _get_nc(reps=1, variant="full"):
    key = ("nc", reps, variant)
    if key not in _CACHE:
        _CACHE[key] = _build_nc(reps, variant)
    return _CACHE[key]


def make_in_maps(z, g0_w, g0_b, g1_w, g1_b, g2_w, g2_b, w2, b2, **_unused):
    import ml_dtypes
    bf16 = np.dtype(ml_dtypes.bfloat16)

    z = np.asarray(z, dtype=np.float32)
    g0_w = np.asarray(g0_w, dtype=np.float32)
    g1_w = np.asarray(g1_w, dtype=np.float32)
    g2_w = np.asarray(g2_w, dtype=np.float32)
    g0_b = np.asarray(g0_b, dtype=np.float32)
    g1_b = np.asarray(g1_b, dtype=np.float32)
    g2_b = np.asarray(g2_b, dtype=np.float32)
    w2 = np.asarray(w2, dtype=np.float32)
    b2 = np.asarray(b2, dtype=np.float32)

    GWX = NK * GATE_H + GATE_H + E
    gwp = np.zeros((KP, GWX), dtype=np.float32)
    for i in range(NK):
        gwp[0:KC, i * GATE_H:(i + 1) * GATE_H] = g0_w[i * KC:(i + 1) * KC]
    gwp[KC, (NK - 1) * GATE_H:NK * GATE_H] = g0_b
    # adjusted biases absorb the ELU "+1" offset of the previous layer
    gwp[0:GATE_H, NK * GATE_H:NK * GATE_H + GATE_H] = g1_w
    gwp[GATE_H, NK * GATE_H:NK * GATE_H + GATE_H] = g1_b - g1_w.sum(axis=0)
    gwp[0:GATE_H, NK * GATE_H + GATE_H:] = g2_w
    gwp[GATE_H, NK * GATE_H + GATE_H:] = g2_b - g2_w.sum(axis=0)

    w2t = np.ascontiguousarray(w2.transpose(1, 0, 2)).reshape(IN_SIZE, E * OUT_SIZE)
    # pair-major packing with a bias row: piece p = concat over K-chunks of
    # pair p's 512 cols; row 96 of the last chunk block = b2 pair cols
    w2p = np.zeros((KP, E // 2 * PW), dtype=np.float32)
    for p in range(E // 2):
        for i in range(NK):
            blk = w2t[i * KC:(i + 1) * KC,
                      2 * p * OUT_SIZE:(2 * p + 2) * OUT_SIZE]
            w2p[0:KC, p * PW + i * 2 * OUT_SIZE:p * PW + (i + 1) * 2 * OUT_SIZE] = blk
        w2p[KC, p * PW + (NK - 1) * 2 * OUT_SIZE:p * PW + NK * 2 * OUT_SIZE] = \
            b2[2 * p:2 * p + 2].reshape(-1)

    shared = {
        "gwp": np.ascontiguousarray(gwp.astype(bf16)),
        "w2p": np.ascontiguousarray(w2p.astype(bf16)),
    }
    maps = []
    for c in range(N_CORES):
        zT = z[c * BL:(c + 1) * BL].T                      # [288, 256]
        zTp = np.ones((KP, NK * BL), dtype=np.float32)
        for i in range(NK):
            zTp[0:KC, i * BL:(i + 1) * BL] = zT[i * KC:(i + 1) * KC]
        maps.append(dict(shared, zTp=np.ascontiguousarray(zTp.astype(bf16))))
    return maps


def unpack_out(res_list):
    full = np.empty((B, OUT_SIZE), dtype=np.float32)
    for c in range(N_CORES):
        packed = res_list[c]["outp"]
        for ch in range(NCH):
            full[c * BL + ch * 128:c * BL + (ch + 1) * 128] = \
                packed[:, ch * OUT_SIZE:(ch + 1) * OUT_SIZE]
    return full


def kernel(**inputs):
    from concourse.bass_utils import run_bass_kernel_spmd

    nc = _get_nc()
    in_maps = make_in_maps(**inputs)
    res = run_bass_kernel_spmd(nc, in_maps, list(range(N_CORES)))
    return unpack_out(res.results)
